# revision 24
# baseline (speedup 1.0000x reference)
"""GCN mean-aggregation (DGL copy_src -> mean by dst) on 8 NeuronCores.

Strategy (dst-sharded, no collectives):
  - Host: edges are assigned to the core owning their dst row (core c owns
    rows [c*12500, (c+1)*12500)).  Within a core, dst nodes form 98 buckets
    of 128; src rows are split into 4 groups of 25000 so gather indices fit
    int16 (dma_gather requirement).  Edges are sorted by
    (bucket-wave, src-group, bucket, src) and each (bucket, group) run is
    padded to a static number of 128-edge tiles (max over the 8 cores), so a
    single program serves all cores.  Pad edges gather a garbage row and are
    masked out by a zero one-hot row (dst_local = 128).
    The embedding table is shipped as bf16 padded to 128 cols (256B rows --
    the dma_gather minimum granularity), and per-node 1/max(indeg,1) is
    precomputed on the host (it depends only on dst).
  - Device (identical program per core):
      * per (wave of 16 buckets) x (src group): batched dma_gather of bf16
        256B rows into SBUF, round-robin over 4 SWDGE queues
      * per edge-tile: one-hot(dst_local) built on DVE (bf16 iota + is_equal)
      * per edge-tile: psum[:, :64] += onehot^T @ msgs  (bf16 matmul, f32 acc)
      * per bucket: out = psum * recip  on the Scalar engine (Copy w/ scale)
  - Host: concatenate the 8 per-core [12500, 64] outputs.
"""

import sys
from contextlib import ExitStack

import numpy as np
from ml_dtypes import bfloat16

sys.path.insert(0, "/opt/trn_rl_repo")

import concourse.bass as bass  # noqa: E402
import concourse.mybir as mybir  # noqa: E402
import concourse.tile as tile  # noqa: E402
from concourse import bacc  # noqa: E402
from concourse.bass_utils import run_bass_kernel_spmd  # noqa: E402

N_NODES = 100000
N_EDGES = 1000000
D_FEAT = 64
D_PAD = 128  # gather element size (256B in bf16)
N_CORES = 8
NODES_PER_CORE = N_NODES // N_CORES  # 12500
BUCKET = 128  # dst nodes per psum bucket (= one-hot free dim)
N_GROUPS = 4  # src-row groups (int16 index range for dma_gather)
WAVE = 16  # buckets per gather wave
N_QUEUES = 4  # SWDGE queues (hw max)


def _schedule(cnt_max, npc, bucket, wave):
    """Static schedule from per-(bucket, group) max edge counts.

    cnt_max: [nb, ngroups] max edge count over cores.
    Returns dict with tiles-per-region, waves, per-call and per-bucket info.
    """
    nb, ngroups = cnt_max.shape
    tbg = -(-cnt_max // 128)  # [nb, ngroups]
    for b in range(nb):
        if tbg[b].sum() == 0:
            tbg[b, 0] = 1  # ensure psum gets reset even for empty buckets

    # full waves of `wave` buckets, then a small trailing wave so the
    # pipeline tail (compute after the last gather) is short
    sizes = [wave] * (nb // wave)
    rem = nb - sum(sizes)
    if rem:
        sizes.append(rem)
    if len(sizes) >= 2 and sizes[-1] > 4:
        sizes[-1] -= 4
        sizes.append(4)
    waves = []
    w0 = 0
    for sz in sizes:
        waves.append(range(w0, w0 + sz))
        w0 += sz
    # region order: (wave, group, bucket-in-wave)
    region_tile0 = np.zeros((nb, ngroups), np.int64)
    calls = []  # [wave][group] -> (tile0, ntiles)
    t = 0
    for wv in waves:
        wcalls = []
        for g in range(ngroups):
            c0 = t
            for b in wv:
                region_tile0[b, g] = t
                t += int(tbg[b, g])
            wcalls.append((c0, t - c0))
        calls.append(wcalls)
    nt = t
    return {
        "tbg": tbg,
        "waves": waves,
        "region_tile0": region_tile0,
        "calls": calls,
        "nt": nt,
    }


def _prep(src, dst, n_nodes, n_cores, npc, bucket, ngroups, wave):
    """Sort/group/pad edges; build per-core device inputs + static schedule."""
    src = np.asarray(src, dtype=np.int64)
    dst = np.asarray(dst, dtype=np.int64)
    gsz = n_nodes // ngroups
    nb = -(-npc // bucket)
    nw = -(-nb // wave)

    core = dst // npc
    b = (dst - core * npc) // bucket
    g = src // gsz
    w = b // wave

    cnt = np.zeros((n_cores, nb, ngroups), np.int64)
    np.add.at(cnt, (core, b, g), 1)
    sched = _schedule(cnt.max(axis=0), npc, bucket, wave)
    tbg, region_tile0, nt = sched["tbg"], sched["region_tile0"], sched["nt"]
    nslot = nt * 128

    # global sort by (core, wave, group, bucket, src)
    key = (((core * nw + w) * ngroups + g) * nb + b)
    order = np.lexsort((src, key))
    ss, ks = src[order], key[order]
    dl = (dst - (core * npc + b * bucket))[order]  # dst_local in [0, bucket)
    gs_sorted = g[order]

    kcnt = np.bincount(ks, minlength=n_cores * nw * ngroups * nb)
    kstart = np.zeros(kcnt.shape[0] + 1, np.int64)
    np.cumsum(kcnt, out=kstart[1:])
    rank = np.arange(ss.shape[0], dtype=np.int64) - kstart[ks]

    slot_base = region_tile0 * 128  # [nb, ngroups], within-core slot offset
    bo, go, co = b[order], gs_sorted, core[order]
    pos = co * nslot + slot_base[bo, go] + rank

    # per-slot group id (for pad values), same for every core
    slot_group = np.zeros(nslot, np.int64)
    for bb in range(nb):
        for gg in range(ngroups):
            t0 = region_tile0[bb, gg] * 128
            slot_group[t0 : t0 + tbg[bb, gg] * 128] = gg

    src_slot = np.tile((slot_group + 1) * gsz - 1, n_cores)  # pad: last row of group
    dstloc = np.full(n_cores * nslot, float(bucket), np.float32)
    src_slot[pos] = ss
    dstloc[pos] = dl.astype(np.float32)

    idx16 = (src_slot - np.tile(slot_group * gsz, n_cores)).astype(np.int16)
    # wrapped index layout: idx j -> partition j%16, col j//16 (device
    # replicates the 16 partitions x8 on-chip)
    idx16 = idx16.reshape(n_cores, nt * 8, 16)
    idxtab = np.ascontiguousarray(idx16.transpose(0, 2, 1))  # [C, 16, nt*8]

    # dst-local table in bucket-major column order: per wave, per bucket, per
    # (group, tile) -- so each bucket's one-hot builds are one contiguous
    # column range (single batched DVE op per bucket).
    col_of_tile = np.zeros(nt, np.int64)
    bucket_col0 = np.zeros(nb, np.int64)
    c = 0
    for wv in sched["waves"]:
        for bb2 in wv:
            bucket_col0[bb2] = c
            for gg in range(ngroups):
                t0 = region_tile0[bb2, gg]
                for j in range(int(tbg[bb2, gg])):
                    col_of_tile[t0 + j] = c
                    c += 1
    sched["bucket_col0"] = bucket_col0
    dst_tiles = dstloc.reshape(n_cores, nt, 128)
    dst_perm = np.empty_like(dst_tiles)
    dst_perm[:, col_of_tile, :] = dst_tiles
    dst_t = np.ascontiguousarray(dst_perm.transpose(0, 2, 1)).astype(
        bfloat16
    )  # [C, 128, nt]

    # per-node 1/max(indegree, 1), laid out [C, 128, nb]: node b*128+p -> (p, b)
    counts = np.bincount(dst, minlength=n_nodes).astype(np.float32)
    rec = 1.0 / np.maximum(counts, 1.0)
    rec_t = np.ones((n_cores, nb * 128), np.float32)
    rec_t[:, :npc] = rec.reshape(n_cores, npc)
    rec_t = np.ascontiguousarray(
        rec_t.reshape(n_cores, nb, 128).transpose(0, 2, 1)
    )  # [C, 128, nb]
    return idxtab, dst_t, rec_t, sched


def _build(n_nodes, d_feat, npc, bucket, ngroups, sched):
    """Build the (per-core) Bass program."""
    gsz = n_nodes // ngroups
    nb = -(-npc // bucket)
    nt = sched["nt"]
    tbg, region_tile0 = sched["tbg"], sched["region_tile0"]
    bucket_col0 = sched["bucket_col0"]
    f32 = mybir.dt.float32
    bf16 = mybir.dt.bfloat16
    i16 = mybir.dt.int16

    nc = bacc.Bacc(
        "TRN2",
        target_bir_lowering=False,
        debug=False,
        num_swdge_queues=N_QUEUES,
    )
    emb = nc.dram_tensor("emb", [n_nodes, D_PAD], bf16, kind="ExternalInput")
    idx_t = nc.dram_tensor("idx_t", [16, nt * 8], i16, kind="ExternalInput")
    dst_t = nc.dram_tensor("dst_t", [128, nt], bf16, kind="ExternalInput")
    rec_t = nc.dram_tensor("rec_t", [128, nb], f32, kind="ExternalInput")
    out = nc.dram_tensor("out", [npc, d_feat], f32, kind="ExternalOutput")

    # process waves largest-first so the pipeline tail (last wave's compute)
    # is as short as possible
    nw = len(sched["waves"])
    wave_nt = [sum(ntl for (_, ntl) in sched["calls"][w]) for w in range(nw)]
    wave_t0 = [sched["calls"][w][0][0] for w in range(nw)]
    wave_order = sorted(range(nw), key=lambda w: -wave_nt[w])

    with tile.TileContext(nc) as tc, ExitStack() as ctx:
        const_p = ctx.enter_context(tc.tile_pool(name="const", bufs=1))
        idx_p = ctx.enter_context(tc.tile_pool(name="idx", bufs=1))
        msgs_p = ctx.enter_context(tc.tile_pool(name="msgs", bufs=3))
        oh_p = ctx.enter_context(tc.tile_pool(name="oh", bufs=6))
        ps_p = ctx.enter_context(tc.tile_pool(name="ps", bufs=4, space="PSUM"))
        outp_p = ctx.enter_context(tc.tile_pool(name="outp", bufs=3))

        recall = idx_p.tile([128, nb], f32)
        nc.sync.dma_start(out=recall[:], in_=rec_t[:, :])

        iota_i = const_p.tile([128, bucket], mybir.dt.int32)
        nc.gpsimd.iota(iota_i[:], pattern=[[1, bucket]], base=0, channel_multiplier=0)
        iota_b = const_p.tile([128, bucket], bf16)
        nc.vector.tensor_copy(out=iota_b[:], in_=iota_i[:])

        def _loads(wvi):
            # per-wave idx load (16 real partitions from HBM, log-doubled to
            # 128 on-chip) + per-wave dst-local columns
            t0w, ntw = wave_t0[wvi], wave_nt[wvi]
            idxw = idx_p.tile([128, ntw * 8], i16, tag=f"idx{wvi}")
            nc.sync.dma_start(
                out=idxw[0:16, :], in_=idx_t[:, t0w * 8 : (t0w + ntw) * 8]
            )
            for p in (16, 32, 64):
                nc.sync.dma_start(out=idxw[p : 2 * p, :], in_=idxw[0:p, :])
            dstw = idx_p.tile([128, ntw], bf16, tag=f"dst{wvi}")
            nc.sync.dma_start(out=dstw[:], in_=dst_t[:, t0w : t0w + ntw])
            return idxw, dstw

        qn = 0
        loaded = {wave_order[0]: _loads(wave_order[0])}
        for wi, wvi in enumerate(wave_order):
            wv = sched["waves"][wvi]
            t0w, ntw = wave_t0[wvi], wave_nt[wvi]
            idxw, dstw = loaded.pop(wvi)

            msgs = {}
            call0 = {}
            for gg in range(ngroups):
                t0, ntl = sched["calls"][wvi][gg]
                call0[gg] = t0
                if ntl == 0:
                    continue
                m = msgs_p.tile([128, ntl * D_PAD], bf16, tag=f"msgs{gg}")
                msgs[gg] = m
                # dma_gather is limited to 1024 indices (8 tiles) per call
                for sc in range(0, ntl, 8):
                    k = min(8, ntl - sc)
                    ts = t0 + sc
                    nc.gpsimd.dma_gather(
                        out_ap=m[:, sc * D_PAD : (sc + k) * D_PAD].rearrange(
                            "p (t e) -> p t e", e=D_PAD
                        ),
                        in_ap=emb[gg * gsz : (gg + 1) * gsz, :],
                        idxs_ap=idxw[:, (ts - t0w) * 8 : (ts - t0w + k) * 8],
                        num_idxs=k * 128,
                        num_idxs_reg=k * 128,
                        elem_size=D_PAD,
                        queue_num=qn,
                    )
                    qn = (qn + 1) % N_QUEUES
            # prefetch next wave's idx/dst ahead of this wave's out-writes
            # in the sync queue
            if wi + 1 < len(wave_order):
                nxt = wave_order[wi + 1]
                loaded[nxt] = _loads(nxt)
            for bb in wv:
                passes = [
                    (gg, region_tile0[bb, gg] + j)
                    for gg in range(ngroups)
                    for j in range(int(tbg[bb, gg]))
                ]
                kb = len(passes)
                c0 = int(bucket_col0[bb]) - t0w
                # all kb one-hots of this bucket in one batched DVE op:
                # onehot[:, i, :] = (iota == dstw[:, c0+i]) for i in 0..kb
                ohb = oh_p.tile([128, kb * bucket], bf16)
                nc.vector.scalar_tensor_tensor(
                    out=ohb[:].rearrange("p (t e) -> p t e", e=bucket),
                    in0=iota_b[:].unsqueeze(1).broadcast_to([128, kb, bucket]),
                    scalar=0.0,
                    in1=dstw[:, c0 : c0 + kb]
                    .unsqueeze(2)
                    .broadcast_to([128, kb, bucket]),
                    op0=mybir.AluOpType.bypass,
                    op1=mybir.AluOpType.is_equal,
                )
                psum = ps_p.tile([bucket, d_feat], f32)
                for i, (gg, t) in enumerate(passes):
                    off = int(t - call0[gg]) * D_PAD
                    nc.tensor.matmul(
                        out=psum[:],
                        lhsT=ohb[:, i * bucket : (i + 1) * bucket],
                        rhs=msgs[gg][:, off : off + d_feat],
                        start=(i == 0),
                        stop=(i == len(passes) - 1),
                    )
                nrows = min(bucket, npc - bb * bucket)
                ot = outp_p.tile([bucket, d_feat], f32)
                nc.scalar.activation(
                    out=ot[:],
                    in_=psum[:],
                    func=mybir.ActivationFunctionType.Copy,
                    scale=recall[:, bb : bb + 1],
                )
                nc.sync.dma_start(
                    out=out[bb * bucket : bb * bucket + nrows, :], in_=ot[:nrows, :]
                )

    nc.compile()
    return nc


_CACHE = {}


def _run(embeddings, src, dst, trace=False, trace_kwargs=None):
    embeddings = np.asarray(embeddings, dtype=np.float32)
    embp = np.zeros((N_NODES, D_PAD), bfloat16)
    embp[:, :D_FEAT] = embeddings.astype(bfloat16)
    idxtab, dst_t, rec_t, sched = _prep(
        src, dst, N_NODES, N_CORES, NODES_PER_CORE, BUCKET, N_GROUPS, WAVE
    )
    key = sched["tbg"].tobytes()
    if key not in _CACHE:
        _CACHE[key] = _build(N_NODES, D_FEAT, NODES_PER_CORE, BUCKET, N_GROUPS, sched)
    nc = _CACHE[key]

    in_maps = [
        {"emb": embp, "idx_t": idxtab[c], "dst_t": dst_t[c], "rec_t": rec_t[c]}
        for c in range(N_CORES)
    ]
    res = run_bass_kernel_spmd(
        nc,
        in_maps,
        core_ids=list(range(N_CORES)),
        trace=trace,
        **(trace_kwargs or {}),
    )
    out = np.concatenate([res.results[c]["out"] for c in range(N_CORES)], axis=0)
    return out, res


def kernel(embeddings, src, dst):
    out, _ = _run(embeddings, src, dst, trace=False)
    return out


# revision 26
# speedup vs baseline: 1.1879x; 1.1879x over previous
"""GCN mean-aggregation (DGL copy_src -> mean by dst) on 8 NeuronCores.

Strategy (dst-sharded, no collectives):
  - Host: edges are assigned to the core owning their dst row (core c owns
    rows [c*12500, (c+1)*12500)).  Within a core, dst nodes form 98 buckets
    of 128; src rows are split into 4 groups of 25000 so gather indices fit
    int16 (dma_gather requirement).  Edges are sorted by
    (bucket-wave, src-group, bucket, src) and each (bucket, group) run is
    padded to a static number of 128-edge tiles (max over the 8 cores), so a
    single program serves all cores.  Pad edges gather a garbage row and are
    masked out by a zero one-hot row (dst_local = 128).
    The embedding table is shipped as bf16 padded to 128 cols (256B rows --
    the dma_gather minimum granularity), and per-node 1/max(indeg,1) is
    precomputed on the host (it depends only on dst).
  - Device (identical program per core):
      * per (wave of 16 buckets) x (src group): batched dma_gather of bf16
        256B rows into SBUF, round-robin over 4 SWDGE queues
      * per edge-tile: one-hot(dst_local) built on DVE (bf16 iota + is_equal)
      * per edge-tile: psum[:, :64] += onehot^T @ msgs  (bf16 matmul, f32 acc)
      * per bucket: out = psum * recip  on the Scalar engine (Copy w/ scale)
  - Host: concatenate the 8 per-core [12500, 64] outputs.
"""

import sys
from contextlib import ExitStack

import numpy as np
from ml_dtypes import bfloat16

sys.path.insert(0, "/opt/trn_rl_repo")

import concourse.bass as bass  # noqa: E402
import concourse.mybir as mybir  # noqa: E402
import concourse.tile as tile  # noqa: E402
from concourse import bacc  # noqa: E402
from concourse.bass_utils import run_bass_kernel_spmd  # noqa: E402

N_NODES = 100000
N_EDGES = 1000000
D_FEAT = 64
D_PAD = 128  # gather element size (256B in bf16)
N_CORES = 8
NODES_PER_CORE = N_NODES // N_CORES  # 12500
BUCKET = 128  # dst nodes per psum bucket (= one-hot free dim)
N_GROUPS = 4  # src-row groups (int16 index range for dma_gather)
WAVE = 16  # buckets per gather wave
N_QUEUES = 4  # SWDGE queues (hw max)


def _schedule(cnt_max, npc, bucket, wave):
    """Static schedule from per-(bucket, group) max edge counts.

    cnt_max: [nb, ngroups] max edge count over cores.
    Returns dict with tiles-per-region, waves, per-call and per-bucket info.
    """
    nb, ngroups = cnt_max.shape
    tbg = -(-cnt_max // 128)  # [nb, ngroups]
    for b in range(nb):
        if tbg[b].sum() == 0:
            tbg[b, 0] = 1  # ensure psum gets reset even for empty buckets

    # full waves of `wave` buckets, then a small trailing wave so the
    # pipeline tail (compute after the last gather) is short
    sizes = [wave] * (nb // wave)
    rem = nb - sum(sizes)
    if rem:
        sizes.append(rem)
    if len(sizes) >= 2 and sizes[-1] > 4:
        sizes[-1] -= 4
        sizes.append(4)
    waves = []
    w0 = 0
    for sz in sizes:
        waves.append(range(w0, w0 + sz))
        w0 += sz
    # region order: (wave, group, bucket-in-wave)
    region_tile0 = np.zeros((nb, ngroups), np.int64)
    calls = []  # [wave][group] -> (tile0, ntiles)
    t = 0
    for wv in waves:
        wcalls = []
        for g in range(ngroups):
            c0 = t
            for b in wv:
                region_tile0[b, g] = t
                t += int(tbg[b, g])
            wcalls.append((c0, t - c0))
        calls.append(wcalls)
    nt = t
    return {
        "tbg": tbg,
        "waves": waves,
        "region_tile0": region_tile0,
        "calls": calls,
        "nt": nt,
    }


def _prep(src, dst, n_nodes, n_cores, npc, bucket, ngroups, wave):
    """Sort/group/pad edges; build per-core device inputs + static schedule."""
    src = np.asarray(src, dtype=np.int64)
    dst = np.asarray(dst, dtype=np.int64)
    gsz = n_nodes // ngroups
    nb = -(-npc // bucket)
    nw = -(-nb // wave)

    core = dst // npc
    b = (dst - core * npc) // bucket
    g = src // gsz
    w = b // wave

    cnt = np.zeros((n_cores, nb, ngroups), np.int64)
    np.add.at(cnt, (core, b, g), 1)
    sched = _schedule(cnt.max(axis=0), npc, bucket, wave)
    tbg, region_tile0, nt = sched["tbg"], sched["region_tile0"], sched["nt"]
    nslot = nt * 128

    # global sort by (core, wave, group, bucket, src)
    key = (((core * nw + w) * ngroups + g) * nb + b)
    order = np.lexsort((src, key))
    ss, ks = src[order], key[order]
    dl = (dst - (core * npc + b * bucket))[order]  # dst_local in [0, bucket)
    gs_sorted = g[order]

    kcnt = np.bincount(ks, minlength=n_cores * nw * ngroups * nb)
    kstart = np.zeros(kcnt.shape[0] + 1, np.int64)
    np.cumsum(kcnt, out=kstart[1:])
    rank = np.arange(ss.shape[0], dtype=np.int64) - kstart[ks]

    slot_base = region_tile0 * 128  # [nb, ngroups], within-core slot offset
    bo, go, co = b[order], gs_sorted, core[order]
    pos = co * nslot + slot_base[bo, go] + rank

    # per-slot group id (for pad values), same for every core
    slot_group = np.zeros(nslot, np.int64)
    for bb in range(nb):
        for gg in range(ngroups):
            t0 = region_tile0[bb, gg] * 128
            slot_group[t0 : t0 + tbg[bb, gg] * 128] = gg

    src_slot = np.tile((slot_group + 1) * gsz - 1, n_cores)  # pad: last row of group
    dstloc = np.full(n_cores * nslot, float(bucket), np.float32)
    src_slot[pos] = ss
    dstloc[pos] = dl.astype(np.float32)

    idx16 = (src_slot - np.tile(slot_group * gsz, n_cores)).astype(np.int16)
    # wrapped index layout: idx j -> partition j%16, col j//16 (device
    # replicates the 16 partitions x8 on-chip)
    idx16 = idx16.reshape(n_cores, nt * 8, 16)
    idxtab = np.ascontiguousarray(idx16.transpose(0, 2, 1))  # [C, 16, nt*8]

    # dst-local table in bucket-major column order: per wave, per bucket, per
    # (group, tile) -- so each bucket's one-hot builds are one contiguous
    # column range (single batched DVE op per bucket).
    col_of_tile = np.zeros(nt, np.int64)
    bucket_col0 = np.zeros(nb, np.int64)
    c = 0
    for wv in sched["waves"]:
        for bb2 in wv:
            bucket_col0[bb2] = c
            for gg in range(ngroups):
                t0 = region_tile0[bb2, gg]
                for j in range(int(tbg[bb2, gg])):
                    col_of_tile[t0 + j] = c
                    c += 1
    sched["bucket_col0"] = bucket_col0
    dst_tiles = dstloc.reshape(n_cores, nt, 128)
    dst_perm = np.empty_like(dst_tiles)
    dst_perm[:, col_of_tile, :] = dst_tiles
    dst_t = np.ascontiguousarray(dst_perm.transpose(0, 2, 1)).astype(
        bfloat16
    )  # [C, 128, nt]

    # per-node 1/max(indegree, 1), laid out [C, 128, nb]: node b*128+p -> (p, b)
    counts = np.bincount(dst, minlength=n_nodes).astype(np.float32)
    rec = 1.0 / np.maximum(counts, 1.0)
    rec_t = np.ones((n_cores, nb * 128), np.float32)
    rec_t[:, :npc] = rec.reshape(n_cores, npc)
    rec_t = np.ascontiguousarray(
        rec_t.reshape(n_cores, nb, 128).transpose(0, 2, 1)
    )  # [C, 128, nb]
    return idxtab, dst_t, rec_t, sched


def _build(n_nodes, d_feat, npc, bucket, ngroups, sched):
    """Build the (per-core) Bass program."""
    gsz = n_nodes // ngroups
    nb = -(-npc // bucket)
    nt = sched["nt"]
    tbg, region_tile0 = sched["tbg"], sched["region_tile0"]
    bucket_col0 = sched["bucket_col0"]
    f32 = mybir.dt.float32
    bf16 = mybir.dt.bfloat16
    i16 = mybir.dt.int16

    nc = bacc.Bacc(
        "TRN2",
        target_bir_lowering=False,
        debug=False,
        num_swdge_queues=N_QUEUES,
    )
    emb = nc.dram_tensor("emb", [n_nodes, D_PAD], bf16, kind="ExternalInput")
    idx_t = nc.dram_tensor("idx_t", [16, nt * 8], i16, kind="ExternalInput")
    dst_t = nc.dram_tensor("dst_t", [128, nt], bf16, kind="ExternalInput")
    rec_t = nc.dram_tensor("rec_t", [128, nb], f32, kind="ExternalInput")
    out = nc.dram_tensor("out", [npc, d_feat], f32, kind="ExternalOutput")

    # process waves largest-first so the pipeline tail (last wave's compute)
    # is as short as possible
    nw = len(sched["waves"])
    wave_nt = [sum(ntl for (_, ntl) in sched["calls"][w]) for w in range(nw)]
    wave_t0 = [sched["calls"][w][0][0] for w in range(nw)]
    wave_order = sorted(range(nw), key=lambda w: -wave_nt[w])

    with tile.TileContext(nc) as tc, ExitStack() as ctx:
        const_p = ctx.enter_context(tc.tile_pool(name="const", bufs=1))
        idx_p = ctx.enter_context(tc.tile_pool(name="idx", bufs=1))
        msgs_p = ctx.enter_context(tc.tile_pool(name="msgs", bufs=3))
        oh_p = ctx.enter_context(tc.tile_pool(name="oh", bufs=6))
        ps_p = ctx.enter_context(tc.tile_pool(name="ps", bufs=4, space="PSUM"))
        outp_p = ctx.enter_context(tc.tile_pool(name="outp", bufs=3))

        recall = idx_p.tile([128, nb], f32)
        nc.sync.dma_start(out=recall[:], in_=rec_t[:, :])

        iota_i = const_p.tile([128, bucket], mybir.dt.int32)
        nc.gpsimd.iota(iota_i[:], pattern=[[1, bucket]], base=0, channel_multiplier=0)
        iota_b = const_p.tile([128, bucket], bf16)
        nc.vector.tensor_copy(out=iota_b[:], in_=iota_i[:])

        qn = 0
        for wi, wvi in enumerate(wave_order):
            wv = sched["waves"][wvi]
            t0w, ntw = wave_t0[wvi], wave_nt[wvi]
            # per-wave idx load (16 real partitions from HBM, replicated x8
            # on-chip) + per-wave dst-local columns
            idxs16 = idx_p.tile([16, ntw * 8], i16, tag=f"idxs{wvi}")
            nc.sync.dma_start(out=idxs16[:], in_=idx_t[:, t0w * 8 : (t0w + ntw) * 8])
            idxw = idx_p.tile([128, ntw * 8], i16, tag=f"idx{wvi}")
            for r in range(8):
                nc.sync.dma_start(out=idxw[r * 16 : (r + 1) * 16, :], in_=idxs16[:])
            dstw = idx_p.tile([128, ntw], bf16, tag=f"dst{wvi}")
            nc.sync.dma_start(out=dstw[:], in_=dst_t[:, t0w : t0w + ntw])

            msgs = {}
            call0 = {}
            for gg in range(ngroups):
                t0, ntl = sched["calls"][wvi][gg]
                call0[gg] = t0
                if ntl == 0:
                    continue
                m = msgs_p.tile([128, ntl * D_PAD], bf16, tag=f"msgs{gg}")
                msgs[gg] = m
                # dma_gather is limited to 1024 indices (8 tiles) per call
                for sc in range(0, ntl, 8):
                    k = min(8, ntl - sc)
                    ts = t0 + sc
                    nc.gpsimd.dma_gather(
                        out_ap=m[:, sc * D_PAD : (sc + k) * D_PAD].rearrange(
                            "p (t e) -> p t e", e=D_PAD
                        ),
                        in_ap=emb[gg * gsz : (gg + 1) * gsz, :],
                        idxs_ap=idxw[:, (ts - t0w) * 8 : (ts - t0w + k) * 8],
                        num_idxs=k * 128,
                        num_idxs_reg=k * 128,
                        elem_size=D_PAD,
                        queue_num=qn,
                    )
                    qn = (qn + 1) % N_QUEUES
            for bb in wv:
                passes = [
                    (gg, region_tile0[bb, gg] + j)
                    for gg in range(ngroups)
                    for j in range(int(tbg[bb, gg]))
                ]
                kb = len(passes)
                c0 = int(bucket_col0[bb]) - t0w
                # all kb one-hots of this bucket in one batched DVE op:
                # onehot[:, i, :] = (iota == dstw[:, c0+i]) for i in 0..kb
                ohb = oh_p.tile([128, kb * bucket], bf16)
                nc.vector.scalar_tensor_tensor(
                    out=ohb[:].rearrange("p (t e) -> p t e", e=bucket),
                    in0=iota_b[:].unsqueeze(1).broadcast_to([128, kb, bucket]),
                    scalar=0.0,
                    in1=dstw[:, c0 : c0 + kb]
                    .unsqueeze(2)
                    .broadcast_to([128, kb, bucket]),
                    op0=mybir.AluOpType.bypass,
                    op1=mybir.AluOpType.is_equal,
                )
                psum = ps_p.tile([bucket, d_feat], f32)
                for i, (gg, t) in enumerate(passes):
                    off = int(t - call0[gg]) * D_PAD
                    nc.tensor.matmul(
                        out=psum[:],
                        lhsT=ohb[:, i * bucket : (i + 1) * bucket],
                        rhs=msgs[gg][:, off : off + d_feat],
                        start=(i == 0),
                        stop=(i == len(passes) - 1),
                    )
                nrows = min(bucket, npc - bb * bucket)
                ot = outp_p.tile([bucket, d_feat], f32)
                nc.scalar.activation(
                    out=ot[:],
                    in_=psum[:],
                    func=mybir.ActivationFunctionType.Copy,
                    scale=recall[:, bb : bb + 1],
                )
                nc.sync.dma_start(
                    out=out[bb * bucket : bb * bucket + nrows, :], in_=ot[:nrows, :]
                )

    nc.compile()
    return nc


_CACHE = {}


def _run(embeddings, src, dst, trace=False, trace_kwargs=None):
    embeddings = np.asarray(embeddings, dtype=np.float32)
    embp = np.zeros((N_NODES, D_PAD), bfloat16)
    embp[:, :D_FEAT] = embeddings.astype(bfloat16)
    idxtab, dst_t, rec_t, sched = _prep(
        src, dst, N_NODES, N_CORES, NODES_PER_CORE, BUCKET, N_GROUPS, WAVE
    )
    key = sched["tbg"].tobytes()
    if key not in _CACHE:
        _CACHE[key] = _build(N_NODES, D_FEAT, NODES_PER_CORE, BUCKET, N_GROUPS, sched)
    nc = _CACHE[key]

    in_maps = [
        {"emb": embp, "idx_t": idxtab[c], "dst_t": dst_t[c], "rec_t": rec_t[c]}
        for c in range(N_CORES)
    ]
    res = run_bass_kernel_spmd(
        nc,
        in_maps,
        core_ids=list(range(N_CORES)),
        trace=trace,
        **(trace_kwargs or {}),
    )
    out = np.concatenate([res.results[c]["out"] for c in range(N_CORES)], axis=0)
    return out, res


def kernel(embeddings, src, dst):
    out, _ = _run(embeddings, src, dst, trace=False)
    return out


# revision 41
# speedup vs baseline: 1.1983x; 1.0087x over previous
"""GCN mean-aggregation (DGL copy_src -> mean by dst) on 8 NeuronCores.

Strategy (dst-sharded, no collectives):
  - Host: edges are assigned to the core owning their dst row (core c owns
    rows [c*12500, (c+1)*12500)).  Within a core, dst nodes form 98 buckets
    of 128; src rows are split into 4 groups of 25000 so gather indices fit
    int16 (dma_gather requirement).  Edges are sorted by
    (bucket-wave, src-group, bucket, src) and each (bucket, group) run is
    padded to a static number of 128-edge tiles (max over the 8 cores), so a
    single program serves all cores.  Pad edges gather a garbage row and are
    masked out by a zero one-hot row (dst_local = 128).
    The embedding table is shipped as bf16 padded to 128 cols (256B rows --
    the dma_gather minimum granularity), and per-node 1/max(indeg,1) is
    precomputed on the host (it depends only on dst).
  - Device (identical program per core):
      * per (wave of 16 buckets) x (src group): batched dma_gather of bf16
        256B rows into SBUF, round-robin over 4 SWDGE queues
      * per edge-tile: one-hot(dst_local) built on DVE (bf16 iota + is_equal)
      * per edge-tile: psum[:, :64] += onehot^T @ msgs  (bf16 matmul, f32 acc)
      * per bucket: out = psum * recip  on the Scalar engine (Copy w/ scale)
  - Host: concatenate the 8 per-core [12500, 64] outputs.
"""

import sys
from contextlib import ExitStack

import numpy as np
from ml_dtypes import bfloat16

sys.path.insert(0, "/opt/trn_rl_repo")

import concourse.bass as bass  # noqa: E402
import concourse.mybir as mybir  # noqa: E402
import concourse.tile as tile  # noqa: E402
from concourse import bacc  # noqa: E402
from concourse.bass_utils import run_bass_kernel_spmd  # noqa: E402

N_NODES = 100000
N_EDGES = 1000000
D_FEAT = 64
D_PAD = 128  # gather element size (256B in bf16)
N_CORES = 8
NODES_PER_CORE = N_NODES // N_CORES  # 12500
BUCKET = 128  # dst nodes per psum bucket (= one-hot free dim)
N_GROUPS = 4  # src-row groups (int16 index range for dma_gather)
WAVE = 16  # buckets per gather wave
N_QUEUES = 4  # SWDGE queues (hw max)


def _schedule(cnt_max, npc, bucket, wave):
    """Static schedule from per-(bucket, group) max edge counts.

    cnt_max: [nb, ngroups] max edge count over cores.
    Returns dict with tiles-per-region, waves, per-call and per-bucket info.
    """
    nb, ngroups = cnt_max.shape
    tbg = -(-cnt_max // 128)  # [nb, ngroups]
    for b in range(nb):
        if tbg[b].sum() == 0:
            tbg[b, 0] = 1  # ensure psum gets reset even for empty buckets

    waves = [range(w, min(w + wave, nb)) for w in range(0, nb, wave)]
    # region order: (wave, group, bucket-in-wave)
    region_tile0 = np.zeros((nb, ngroups), np.int64)
    calls = []  # [wave][group] -> (tile0, ntiles)
    t = 0
    for wv in waves:
        wcalls = []
        for g in range(ngroups):
            c0 = t
            for b in wv:
                region_tile0[b, g] = t
                t += int(tbg[b, g])
            wcalls.append((c0, t - c0))
        calls.append(wcalls)
    nt = t
    return {
        "tbg": tbg,
        "waves": waves,
        "region_tile0": region_tile0,
        "calls": calls,
        "nt": nt,
    }


def _prep(src, dst, n_nodes, n_cores, npc, bucket, ngroups, wave):
    """Sort/group/pad edges; build per-core device inputs + static schedule."""
    src = np.asarray(src, dtype=np.int64)
    dst = np.asarray(dst, dtype=np.int64)
    gsz = n_nodes // ngroups
    nb = -(-npc // bucket)
    nw = -(-nb // wave)

    core = dst // npc
    b = (dst - core * npc) // bucket
    g = src // gsz
    w = b // wave

    cnt = np.zeros((n_cores, nb, ngroups), np.int64)
    np.add.at(cnt, (core, b, g), 1)
    sched = _schedule(cnt.max(axis=0), npc, bucket, wave)
    tbg, region_tile0, nt = sched["tbg"], sched["region_tile0"], sched["nt"]
    nslot = nt * 128

    # global sort by (core, wave, group, bucket, src)
    key = (((core * nw + w) * ngroups + g) * nb + b)
    order = np.lexsort((src, key))
    ss, ks = src[order], key[order]
    dl = (dst - (core * npc + b * bucket))[order]  # dst_local in [0, bucket)
    gs_sorted = g[order]

    kcnt = np.bincount(ks, minlength=n_cores * nw * ngroups * nb)
    kstart = np.zeros(kcnt.shape[0] + 1, np.int64)
    np.cumsum(kcnt, out=kstart[1:])
    rank = np.arange(ss.shape[0], dtype=np.int64) - kstart[ks]

    slot_base = region_tile0 * 128  # [nb, ngroups], within-core slot offset
    bo, go, co = b[order], gs_sorted, core[order]
    pos = co * nslot + slot_base[bo, go] + rank

    # per-slot group id (for pad values), same for every core
    slot_group = np.zeros(nslot, np.int64)
    for bb in range(nb):
        for gg in range(ngroups):
            t0 = region_tile0[bb, gg] * 128
            slot_group[t0 : t0 + tbg[bb, gg] * 128] = gg

    src_slot = np.tile((slot_group + 1) * gsz - 1, n_cores)  # pad: last row of group
    dstloc = np.full(n_cores * nslot, float(bucket), np.float32)
    src_slot[pos] = ss
    dstloc[pos] = dl.astype(np.float32)

    idx16 = (src_slot - np.tile(slot_group * gsz, n_cores)).astype(np.int16)
    # wrapped index layout: idx j -> partition j%16, col j//16 (x8 replicas)
    idx16 = idx16.reshape(n_cores, nt * 8, 16)
    idxtab = np.ascontiguousarray(idx16.transpose(0, 2, 1))  # [C, 16, nt*8]
    idxtab = np.tile(idxtab, (1, 8, 1))  # [C, 128, nt*8]

    # dst-local table in bucket-major column order: per wave, per bucket, per
    # (group, tile) -- so each bucket's one-hot builds are one contiguous
    # column range (single batched DVE op per bucket).
    col_of_tile = np.zeros(nt, np.int64)
    bucket_col0 = np.zeros(nb, np.int64)
    c = 0
    for wv in sched["waves"]:
        for bb2 in wv:
            bucket_col0[bb2] = c
            for gg in range(ngroups):
                t0 = region_tile0[bb2, gg]
                for j in range(int(tbg[bb2, gg])):
                    col_of_tile[t0 + j] = c
                    c += 1
    sched["bucket_col0"] = bucket_col0
    dst_tiles = dstloc.reshape(n_cores, nt, 128)
    dst_perm = np.empty_like(dst_tiles)
    dst_perm[:, col_of_tile, :] = dst_tiles
    dst_t = np.ascontiguousarray(dst_perm.transpose(0, 2, 1)).astype(
        bfloat16
    )  # [C, 128, nt]

    # per-node 1/max(indegree, 1), laid out [C, 128, nb]: node b*128+p -> (p, b)
    counts = np.bincount(dst, minlength=n_nodes).astype(np.float32)
    rec = 1.0 / np.maximum(counts, 1.0)
    rec_t = np.ones((n_cores, nb * 128), np.float32)
    rec_t[:, :npc] = rec.reshape(n_cores, npc)
    rec_t = np.ascontiguousarray(
        rec_t.reshape(n_cores, nb, 128).transpose(0, 2, 1)
    )  # [C, 128, nb]
    return idxtab, dst_t, rec_t, sched


def _build(n_nodes, d_feat, npc, bucket, ngroups, sched):
    """Build the (per-core) Bass program."""
    gsz = n_nodes // ngroups
    nb = -(-npc // bucket)
    nt = sched["nt"]
    tbg, region_tile0 = sched["tbg"], sched["region_tile0"]
    bucket_col0 = sched["bucket_col0"]
    f32 = mybir.dt.float32
    bf16 = mybir.dt.bfloat16
    i16 = mybir.dt.int16

    nc = bacc.Bacc(
        "TRN2",
        target_bir_lowering=False,
        debug=False,
        num_swdge_queues=N_QUEUES,
    )
    emb = nc.dram_tensor("emb", [n_nodes, D_PAD], bf16, kind="ExternalInput")
    idx_t = nc.dram_tensor("idx_t", [128, nt * 8], i16, kind="ExternalInput")
    dst_t = nc.dram_tensor("dst_t", [128, nt], bf16, kind="ExternalInput")
    rec_t = nc.dram_tensor("rec_t", [128, nb], f32, kind="ExternalInput")
    out = nc.dram_tensor("out", [npc, d_feat], f32, kind="ExternalOutput")

    # process waves largest-first so the pipeline tail (last wave's compute)
    # is as short as possible
    nw = len(sched["waves"])
    wave_nt = [sum(ntl for (_, ntl) in sched["calls"][w]) for w in range(nw)]
    wave_t0 = [sched["calls"][w][0][0] for w in range(nw)]
    wave_order = sorted(range(nw), key=lambda w: -wave_nt[w])

    with tile.TileContext(nc) as tc, ExitStack() as ctx:
        const_p = ctx.enter_context(tc.tile_pool(name="const", bufs=1))
        idx_p = ctx.enter_context(tc.tile_pool(name="idx", bufs=1))
        msgs_p = ctx.enter_context(tc.tile_pool(name="msgs", bufs=3))
        oh_p = ctx.enter_context(tc.tile_pool(name="oh", bufs=6))
        ps_p = ctx.enter_context(tc.tile_pool(name="ps", bufs=4, space="PSUM"))
        outp_p = ctx.enter_context(tc.tile_pool(name="outp", bufs=3))

        recall = idx_p.tile([128, nb], f32)
        nc.sync.dma_start(out=recall[:], in_=rec_t[:, :])

        iota_i = const_p.tile([128, bucket], mybir.dt.int32)
        nc.gpsimd.iota(iota_i[:], pattern=[[1, bucket]], base=0, channel_multiplier=0)
        iota_b = const_p.tile([128, bucket], bf16)
        nc.vector.tensor_copy(out=iota_b[:], in_=iota_i[:])

        qn = 0
        for wi, wvi in enumerate(wave_order):
            wv = sched["waves"][wvi]
            t0w, ntw = wave_t0[wvi], wave_nt[wvi]
            # per-wave idx/dst loads: single slice DMAs, so the first wave's
            # gathers start as soon as its slice lands
            idxw = idx_p.tile([128, ntw * 8], i16, tag=f"idx{wvi}")
            nc.sync.dma_start(out=idxw[:], in_=idx_t[:, t0w * 8 : (t0w + ntw) * 8])
            dstw = idx_p.tile([128, ntw], bf16, tag=f"dst{wvi}")
            nc.sync.dma_start(out=dstw[:], in_=dst_t[:, t0w : t0w + ntw])

            msgs = {}
            call0 = {}
            for gg in range(ngroups):
                t0, ntl = sched["calls"][wvi][gg]
                call0[gg] = t0
                if ntl == 0:
                    continue
                m = msgs_p.tile([128, ntl * D_PAD], bf16, tag=f"msgs{gg}")
                msgs[gg] = m
                # dma_gather is limited to 1024 indices (8 tiles) per call
                for sc in range(0, ntl, 8):
                    k = min(8, ntl - sc)
                    ts = t0 + sc
                    nc.gpsimd.dma_gather(
                        out_ap=m[:, sc * D_PAD : (sc + k) * D_PAD].rearrange(
                            "p (t e) -> p t e", e=D_PAD
                        ),
                        in_ap=emb[gg * gsz : (gg + 1) * gsz, :],
                        idxs_ap=idxw[:, (ts - t0w) * 8 : (ts - t0w + k) * 8],
                        num_idxs=k * 128,
                        num_idxs_reg=k * 128,
                        elem_size=D_PAD,
                        queue_num=qn,
                    )
                    qn = (qn + 1) % N_QUEUES
            for bb in wv:
                passes = [
                    (gg, region_tile0[bb, gg] + j)
                    for gg in range(ngroups)
                    for j in range(int(tbg[bb, gg]))
                ]
                kb = len(passes)
                c0 = int(bucket_col0[bb]) - t0w
                # all kb one-hots of this bucket in one batched DVE op:
                # onehot[:, i, :] = (iota == dstw[:, c0+i]) for i in 0..kb
                ohb = oh_p.tile([128, kb * bucket], bf16)
                nc.vector.scalar_tensor_tensor(
                    out=ohb[:].rearrange("p (t e) -> p t e", e=bucket),
                    in0=iota_b[:].unsqueeze(1).broadcast_to([128, kb, bucket]),
                    scalar=0.0,
                    in1=dstw[:, c0 : c0 + kb]
                    .unsqueeze(2)
                    .broadcast_to([128, kb, bucket]),
                    op0=mybir.AluOpType.bypass,
                    op1=mybir.AluOpType.is_equal,
                )
                psum = ps_p.tile([bucket, d_feat], f32)
                for i, (gg, t) in enumerate(passes):
                    off = int(t - call0[gg]) * D_PAD
                    nc.tensor.matmul(
                        out=psum[:],
                        lhsT=ohb[:, i * bucket : (i + 1) * bucket],
                        rhs=msgs[gg][:, off : off + d_feat],
                        start=(i == 0),
                        stop=(i == len(passes) - 1),
                    )
                nrows = min(bucket, npc - bb * bucket)
                ot = outp_p.tile([bucket, d_feat], f32)
                nc.scalar.activation(
                    out=ot[:],
                    in_=psum[:],
                    func=mybir.ActivationFunctionType.Copy,
                    scale=recall[:, bb : bb + 1],
                )
                nc.sync.dma_start(
                    out=out[bb * bucket : bb * bucket + nrows, :], in_=ot[:nrows, :]
                )

    nc.compile()
    return nc


_CACHE = {}


def _run(embeddings, src, dst, trace=False, trace_kwargs=None):
    embeddings = np.asarray(embeddings, dtype=np.float32)
    embp = np.zeros((N_NODES, D_PAD), bfloat16)
    embp[:, :D_FEAT] = embeddings.astype(bfloat16)
    idxtab, dst_t, rec_t, sched = _prep(
        src, dst, N_NODES, N_CORES, NODES_PER_CORE, BUCKET, N_GROUPS, WAVE
    )
    key = sched["tbg"].tobytes()
    if key not in _CACHE:
        _CACHE[key] = _build(N_NODES, D_FEAT, NODES_PER_CORE, BUCKET, N_GROUPS, sched)
    nc = _CACHE[key]

    in_maps = [
        {"emb": embp, "idx_t": idxtab[c], "dst_t": dst_t[c], "rec_t": rec_t[c]}
        for c in range(N_CORES)
    ]
    res = run_bass_kernel_spmd(
        nc,
        in_maps,
        core_ids=list(range(N_CORES)),
        trace=trace,
        **(trace_kwargs or {}),
    )
    out = np.concatenate([res.results[c]["out"] for c in range(N_CORES)], axis=0)
    return out, res


def kernel(embeddings, src, dst):
    out, _ = _run(embeddings, src, dst, trace=False)
    return out


# revision 43
# speedup vs baseline: 1.2343x; 1.0301x over previous
"""GCN mean-aggregation (DGL copy_src -> mean by dst) on 8 NeuronCores.

Strategy (dst-sharded, no collectives):
  - Host: edges are assigned to the core owning their dst row (core c owns
    rows [c*12500, (c+1)*12500)).  Within a core, dst nodes form 98 buckets
    of 128; src rows are split into 4 groups of 25000 so gather indices fit
    int16 (dma_gather requirement).  Edges are sorted by
    (bucket-wave, src-group, bucket, src) and each (bucket, group) run is
    padded to a static number of 128-edge tiles (max over the 8 cores), so a
    single program serves all cores.  Pad edges gather a garbage row and are
    masked out by a zero one-hot row (dst_local = 128).
    The embedding table is shipped as bf16 padded to 128 cols (256B rows --
    the dma_gather minimum granularity), and per-node 1/max(indeg,1) is
    precomputed on the host (it depends only on dst).
  - Device (identical program per core):
      * per (wave of 16 buckets) x (src group): batched dma_gather of bf16
        256B rows into SBUF, round-robin over 4 SWDGE queues
      * per edge-tile: one-hot(dst_local) built on DVE (bf16 iota + is_equal)
      * per edge-tile: psum[:, :64] += onehot^T @ msgs  (bf16 matmul, f32 acc)
      * per bucket: out = psum * recip  on the Scalar engine (Copy w/ scale)
  - Host: concatenate the 8 per-core [12500, 64] outputs.
"""

import sys
from contextlib import ExitStack

import numpy as np
from ml_dtypes import bfloat16

sys.path.insert(0, "/opt/trn_rl_repo")

import concourse.bass as bass  # noqa: E402
import concourse.mybir as mybir  # noqa: E402
import concourse.tile as tile  # noqa: E402
from concourse import bacc  # noqa: E402
from concourse.bass_utils import run_bass_kernel_spmd  # noqa: E402

N_NODES = 100000
N_EDGES = 1000000
D_FEAT = 64
D_PAD = 128  # gather element size (256B in bf16)
N_CORES = 8
NODES_PER_CORE = N_NODES // N_CORES  # 12500
BUCKET = 128  # dst nodes per psum bucket (= one-hot free dim)
N_GROUPS = 4  # src-row groups (int16 index range for dma_gather)
WAVE = 16  # buckets per gather wave
N_QUEUES = 4  # SWDGE queues (hw max)


def _schedule(cnt_max, npc, bucket, wave):
    """Static schedule from per-(bucket, group) max edge counts.

    cnt_max: [nb, ngroups] max edge count over cores.
    Returns dict with tiles-per-region, waves, per-call and per-bucket info.
    """
    nb, ngroups = cnt_max.shape
    tbg = -(-cnt_max // 128)  # [nb, ngroups]
    for b in range(nb):
        if tbg[b].sum() == 0:
            tbg[b, 0] = 1  # ensure psum gets reset even for empty buckets

    waves = [range(w, min(w + wave, nb)) for w in range(0, nb, wave)]
    # region order: (wave, group, bucket-in-wave)
    region_tile0 = np.zeros((nb, ngroups), np.int64)
    calls = []  # [wave][group] -> (tile0, ntiles)
    t = 0
    for wv in waves:
        wcalls = []
        for g in range(ngroups):
            c0 = t
            for b in wv:
                region_tile0[b, g] = t
                t += int(tbg[b, g])
            wcalls.append((c0, t - c0))
        calls.append(wcalls)
    nt = t
    return {
        "tbg": tbg,
        "waves": waves,
        "region_tile0": region_tile0,
        "calls": calls,
        "nt": nt,
    }


def _prep(src, dst, n_nodes, n_cores, npc, bucket, ngroups, wave):
    """Sort/group/pad edges; build per-core device inputs + static schedule."""
    src = np.asarray(src, dtype=np.int64)
    dst = np.asarray(dst, dtype=np.int64)
    gsz = n_nodes // ngroups
    nb = -(-npc // bucket)
    nw = -(-nb // wave)

    core = dst // npc
    b = (dst - core * npc) // bucket
    g = src // gsz
    w = b // wave

    cnt = np.zeros((n_cores, nb, ngroups), np.int64)
    np.add.at(cnt, (core, b, g), 1)
    sched = _schedule(cnt.max(axis=0), npc, bucket, wave)
    tbg, region_tile0, nt = sched["tbg"], sched["region_tile0"], sched["nt"]
    nslot = nt * 128

    # global sort by (core, wave, group, bucket, src)
    key = (((core * nw + w) * ngroups + g) * nb + b)
    order = np.lexsort((src, key))
    ss, ks = src[order], key[order]
    dl = (dst - (core * npc + b * bucket))[order]  # dst_local in [0, bucket)
    gs_sorted = g[order]

    kcnt = np.bincount(ks, minlength=n_cores * nw * ngroups * nb)
    kstart = np.zeros(kcnt.shape[0] + 1, np.int64)
    np.cumsum(kcnt, out=kstart[1:])
    rank = np.arange(ss.shape[0], dtype=np.int64) - kstart[ks]

    slot_base = region_tile0 * 128  # [nb, ngroups], within-core slot offset
    bo, go, co = b[order], gs_sorted, core[order]
    pos = co * nslot + slot_base[bo, go] + rank

    # per-slot group id (for pad values), same for every core
    slot_group = np.zeros(nslot, np.int64)
    for bb in range(nb):
        for gg in range(ngroups):
            t0 = region_tile0[bb, gg] * 128
            slot_group[t0 : t0 + tbg[bb, gg] * 128] = gg

    src_slot = np.tile((slot_group + 1) * gsz - 1, n_cores)  # pad: last row of group
    dstloc = np.full(n_cores * nslot, float(bucket), np.float32)
    src_slot[pos] = ss
    dstloc[pos] = dl.astype(np.float32)

    idx16 = (src_slot - np.tile(slot_group * gsz, n_cores)).astype(np.int16)
    # wrapped index layout: idx j -> partition j%16, col j//16 (x8 replicas)
    idx16 = idx16.reshape(n_cores, nt * 8, 16)
    idxtab = np.ascontiguousarray(idx16.transpose(0, 2, 1))  # [C, 16, nt*8]
    idxtab = np.tile(idxtab, (1, 8, 1))  # [C, 128, nt*8]

    # dst-local table in bucket-major column order: per wave, per bucket, per
    # (group, tile) -- so each bucket's one-hot builds are one contiguous
    # column range (single batched DVE op per bucket).
    col_of_tile = np.zeros(nt, np.int64)
    bucket_col0 = np.zeros(nb, np.int64)
    c = 0
    for wv in sched["waves"]:
        for bb2 in wv:
            bucket_col0[bb2] = c
            for gg in range(ngroups):
                t0 = region_tile0[bb2, gg]
                for j in range(int(tbg[bb2, gg])):
                    col_of_tile[t0 + j] = c
                    c += 1
    sched["bucket_col0"] = bucket_col0
    dst_tiles = dstloc.reshape(n_cores, nt, 128)
    dst_perm = np.empty_like(dst_tiles)
    dst_perm[:, col_of_tile, :] = dst_tiles
    dst_t = np.ascontiguousarray(dst_perm.transpose(0, 2, 1)).astype(
        bfloat16
    )  # [C, 128, nt]

    # per-node 1/max(indegree, 1), laid out [C, 128, nb]: node b*128+p -> (p, b)
    counts = np.bincount(dst, minlength=n_nodes).astype(np.float32)
    rec = 1.0 / np.maximum(counts, 1.0)
    rec_t = np.ones((n_cores, nb * 128), np.float32)
    rec_t[:, :npc] = rec.reshape(n_cores, npc)
    rec_t = np.ascontiguousarray(
        rec_t.reshape(n_cores, nb, 128).transpose(0, 2, 1)
    )  # [C, 128, nb]
    return idxtab, dst_t, rec_t, sched


def _build(n_nodes, d_feat, npc, bucket, ngroups, sched):
    """Build the (per-core) Bass program."""
    gsz = n_nodes // ngroups
    nb = -(-npc // bucket)
    nt = sched["nt"]
    tbg, region_tile0 = sched["tbg"], sched["region_tile0"]
    bucket_col0 = sched["bucket_col0"]
    f32 = mybir.dt.float32
    bf16 = mybir.dt.bfloat16
    i16 = mybir.dt.int16

    nc = bacc.Bacc(
        "TRN2",
        target_bir_lowering=False,
        debug=False,
        num_swdge_queues=N_QUEUES,
    )
    emb = nc.dram_tensor("emb", [n_nodes, D_PAD], bf16, kind="ExternalInput")
    idx_t = nc.dram_tensor("idx_t", [128, nt * 8], i16, kind="ExternalInput")
    dst_t = nc.dram_tensor("dst_t", [128, nt], bf16, kind="ExternalInput")
    rec_t = nc.dram_tensor("rec_t", [128, nb], f32, kind="ExternalInput")
    out = nc.dram_tensor("out", [npc, d_feat], f32, kind="ExternalOutput")

    # process waves largest-first so the pipeline tail (last wave's compute)
    # is as short as possible
    nw = len(sched["waves"])
    wave_nt = [sum(ntl for (_, ntl) in sched["calls"][w]) for w in range(nw)]
    wave_t0 = [sched["calls"][w][0][0] for w in range(nw)]
    wave_order = sorted(range(nw), key=lambda w: -wave_nt[w])

    with tile.TileContext(nc) as tc, ExitStack() as ctx:
        const_p = ctx.enter_context(tc.tile_pool(name="const", bufs=1))
        idx_p = ctx.enter_context(tc.tile_pool(name="idx", bufs=1))
        msgs_p = ctx.enter_context(tc.tile_pool(name="msgs", bufs=3))
        oh_p = ctx.enter_context(tc.tile_pool(name="oh", bufs=6))
        ps_p = ctx.enter_context(tc.tile_pool(name="ps", bufs=4, space="PSUM"))
        outp_p = ctx.enter_context(tc.tile_pool(name="outp", bufs=3))

        recall = idx_p.tile([128, nb], f32)
        nc.sync.dma_start(out=recall[:], in_=rec_t[:, :])

        iota_i = const_p.tile([128, bucket], mybir.dt.int32)
        nc.gpsimd.iota(iota_i[:], pattern=[[1, bucket]], base=0, channel_multiplier=0)
        iota_b = const_p.tile([128, bucket], bf16)
        nc.vector.tensor_copy(out=iota_b[:], in_=iota_i[:])

        qn = 0
        for wi, wvi in enumerate(wave_order):
            wv = sched["waves"][wvi]
            t0w, ntw = wave_t0[wvi], wave_nt[wvi]
            # per-wave idx/dst loads: single slice DMAs, so the first wave's
            # gathers start as soon as its slice lands
            idxw = idx_p.tile([128, ntw * 8], i16, tag=f"idx{wvi}")
            nc.sync.dma_start(out=idxw[:], in_=idx_t[:, t0w * 8 : (t0w + ntw) * 8])
            dstw = idx_p.tile([128, ntw], bf16, tag=f"dst{wvi}")
            nc.sync.dma_start(out=dstw[:], in_=dst_t[:, t0w : t0w + ntw])

            msgs = {}
            call0 = {}
            for gg in range(ngroups):
                t0, ntl = sched["calls"][wvi][gg]
                call0[gg] = t0
                if ntl == 0:
                    continue
                m = msgs_p.tile([128, ntl * D_PAD], bf16, tag=f"msgs{gg}")
                msgs[gg] = m
                # dma_gather is limited to 1024 indices (8 tiles) per call
                for sc in range(0, ntl, 8):
                    k = min(8, ntl - sc)
                    ts = t0 + sc
                    nc.gpsimd.dma_gather(
                        out_ap=m[:, sc * D_PAD : (sc + k) * D_PAD].rearrange(
                            "p (t e) -> p t e", e=D_PAD
                        ),
                        in_ap=emb[gg * gsz : (gg + 1) * gsz, :],
                        idxs_ap=idxw[:, (ts - t0w) * 8 : (ts - t0w + k) * 8],
                        num_idxs=k * 128,
                        num_idxs_reg=k * 128,
                        elem_size=D_PAD,
                        queue_num=qn,
                    )
                    qn = (qn + 1) % N_QUEUES
            for bb in wv:
                passes = [
                    (gg, region_tile0[bb, gg] + j)
                    for gg in range(ngroups)
                    for j in range(int(tbg[bb, gg]))
                ]
                kb = len(passes)
                c0 = int(bucket_col0[bb]) - t0w
                # all kb one-hots of this bucket in one batched DVE op:
                # onehot[:, i, :] = (iota == dstw[:, c0+i]) for i in 0..kb
                ohb = oh_p.tile([128, kb * bucket], bf16)
                nc.vector.scalar_tensor_tensor(
                    out=ohb[:].rearrange("p (t e) -> p t e", e=bucket),
                    in0=iota_b[:].unsqueeze(1).broadcast_to([128, kb, bucket]),
                    scalar=0.0,
                    in1=dstw[:, c0 : c0 + kb]
                    .unsqueeze(2)
                    .broadcast_to([128, kb, bucket]),
                    op0=mybir.AluOpType.bypass,
                    op1=mybir.AluOpType.is_equal,
                )
                psum = ps_p.tile([bucket, d_feat], f32)
                for i, (gg, t) in enumerate(passes):
                    off = int(t - call0[gg]) * D_PAD
                    nc.tensor.matmul(
                        out=psum[:],
                        lhsT=ohb[:, i * bucket : (i + 1) * bucket],
                        rhs=msgs[gg][:, off : off + d_feat],
                        start=(i == 0),
                        stop=(i == len(passes) - 1),
                    )
                nrows = min(bucket, npc - bb * bucket)
                ot = outp_p.tile([bucket, d_feat], f32)
                nc.scalar.activation(
                    out=ot[:],
                    in_=psum[:],
                    func=mybir.ActivationFunctionType.Copy,
                    scale=recall[:, bb : bb + 1],
                )
                nc.sync.dma_start(
                    out=out[bb * bucket : bb * bucket + nrows, :], in_=ot[:nrows, :]
                )

    nc.compile()
    return nc


_CACHE = {}


def _run(embeddings, src, dst, trace=False, trace_kwargs=None):
    embeddings = np.asarray(embeddings, dtype=np.float32)
    embp = np.zeros((N_NODES, D_PAD), bfloat16)
    embp[:, :D_FEAT] = embeddings.astype(bfloat16)
    idxtab, dst_t, rec_t, sched = _prep(
        src, dst, N_NODES, N_CORES, NODES_PER_CORE, BUCKET, N_GROUPS, WAVE
    )
    key = sched["tbg"].tobytes()
    if key not in _CACHE:
        _CACHE[key] = _build(N_NODES, D_FEAT, NODES_PER_CORE, BUCKET, N_GROUPS, sched)
    nc = _CACHE[key]

    in_maps = [
        {"emb": embp, "idx_t": idxtab[c], "dst_t": dst_t[c], "rec_t": rec_t[c]}
        for c in range(N_CORES)
    ]
    res = run_bass_kernel_spmd(
        nc,
        in_maps,
        core_ids=list(range(N_CORES)),
        trace=trace,
        **(trace_kwargs or {}),
    )
    out = np.concatenate([res.results[c]["out"] for c in range(N_CORES)], axis=0)
    return out, res


def kernel(embeddings, src, dst):
    out, _ = _run(embeddings, src, dst, trace=False)
    return out
